# revision 20
# baseline (speedup 1.0000x reference)
"""Multi-head attention (B=2, S=2048, D=1024, H=16, dh=64) on 8 TRN2 NeuronCores.

Sharding: data-parallel over batch (2) x tensor-parallel over heads (4 per core).
Core c handles batch c//4 and heads [4*(c%4), 4*(c%4)+4). Each core computes a
partial output (its heads' contribution through Wo); the host sums the 4 partials
per batch and adds bo (the unshard step for a sum-sharded tensor).

Matmuls run in bf16 with f32 PSUM accumulation. All DRAM operands are pre-tiled
on host so every DMA line is >=4KB contiguous per partition (small-line DMA ran
at ~140GB/s and dominated the startup critical path).

The PV matmul carries an extra ones-column in the stationary operand so the
softmax denominator falls out of the same accumulation for free; bv is pre-added
to V. Normalization is one approx-reciprocal (custom DVE op, needs base
partition 0) + gpsimd partition broadcast + DVE multiply. The V projection is
computed transposed (512-wide matmuls, full PE rate) and moved to its natural
layout with PE transposes against a host-supplied identity. The output
projection for chunk c is emitted after chunk c+1's attention so the in-order
PE queue never waits on the normalize chain, and the output DMA (bf16) overlaps
remaining compute.
"""

import sys

if "/opt/trn_rl_repo" not in sys.path:
    sys.path.insert(0, "/opt/trn_rl_repo")

import ml_dtypes
import numpy as np

import concourse.bass as bass
import concourse.mybir as mybir
import concourse.tile as tile
from concourse import bacc, bass_utils
from concourse.bass import ts

# Problem constants (hardcoded per contract)
B, S, D = 2, 2048, 1024
H, DH = 16, 64            # total heads, head dim
HC = 4                    # heads per core
DHC = HC * DH             # 256 projected dims per core
NCORES = 8
P = 128
CH = 512                  # query-chunk for attention / projection sub-chunk
NCH = S // CH             # 4
TT = S // P               # 16 key tiles
KO = D // P               # 8 contraction tiles for projections

f32 = mybir.dt.float32
bf16 = mybir.dt.bfloat16
EXP = mybir.ActivationFunctionType.Exp

_compiled = None          # cached nc across calls
last_results = None       # BassKernelResults of the most recent run (for profiling)


def _build():
    nc = bacc.Bacc("TRN2", target_bir_lowering=False, debug=False)

    # Per-core DRAM parameters, pre-tiled on host:
    #   activations x{q,k,v}: [NCH, P, KO, CH]  (per partition: KO*CH*2B = 8KB)
    #   weights w{q,k,v}:     [P, KO, DHC]      (4KB lines)
    #   wo:                   [P, DHC//P, D]    (4KB lines)
    xq_d = nc.dram_tensor("xq", [NCH, P, KO, CH], bf16, kind="ExternalInput")
    xk_d = nc.dram_tensor("xk", [NCH, P, KO, CH], bf16, kind="ExternalInput")
    xv_d = nc.dram_tensor("xv", [NCH, P, KO, CH], bf16, kind="ExternalInput")
    wq = nc.dram_tensor("wq", [P, KO, DHC], bf16, kind="ExternalInput")
    wk = nc.dram_tensor("wk", [P, KO, DHC], bf16, kind="ExternalInput")
    wv = nc.dram_tensor("wv", [P, KO, DHC], bf16, kind="ExternalInput")
    wo = nc.dram_tensor("wo", [P, DHC // P, D], bf16, kind="ExternalInput")
    bq = nc.dram_tensor("bq", [DHC], f32, kind="ExternalInput")
    bk = nc.dram_tensor("bk", [DHC], f32, kind="ExternalInput")
    bv = nc.dram_tensor("bv", [DHC], f32, kind="ExternalInput")
    ident = nc.dram_tensor("ident", [P, P], bf16, kind="ExternalInput")
    out = nc.dram_tensor("out", [S, D], bf16, kind="ExternalOutput")

    with tile.TileContext(nc) as tc:
        with (
            tc.tile_pool(name="weights", bufs=1) as wpool,
            tc.tile_pool(name="acts", bufs=1) as apool,
            tc.tile_pool(name="xin", bufs=3) as xpool,
            tc.tile_pool(name="pt", bufs=6) as ptpool,
            tc.tile_pool(name="small", bufs=2) as spool,
            tc.tile_pool(name="outs", bufs=3) as opool,
            tc.tile_pool(name="shared_ps", bufs=2, space="PSUM") as shared_ps,
            tc.tile_pool(name="l_ps", bufs=3, space="PSUM") as l_ps,
        ):
            # ---- weights / activation tiles ----
            wq_sb = wpool.tile([P, KO, DHC], bf16, tag="wq")
            wk_sb = wpool.tile([P, KO, DHC], bf16, tag="wk")
            wv_sb = wpool.tile([P, KO, DHC], bf16, tag="wv")
            wo_sb = wpool.tile([P, DHC // P, D], bf16, tag="wo")
            id_sb = wpool.tile([P, P], bf16, tag="ident")
            bq_sb = wpool.tile([P, 2], f32, tag="bq")
            bk_sb = wpool.tile([P, 2], f32, tag="bk")
            bv_row = wpool.tile([P, DHC], f32, tag="bv_row")
            bv_bc = wpool.tile([P, DHC], f32, tag="bv_bc")
            bv_heads = bv_bc[:, :].rearrange("p (h c) -> p h c", c=DH)

            # q^T/k^T: [P, m, S] where projected dim r lives at (r % 128, r // 128)
            q_sb = apool.tile([P, 2, S], bf16, tag="q")
            k_sb = apool.tile([P, 2, S], bf16, tag="k")
            # v natural + ones column per head: [P, tt, 4*65 (+63 pad so every
            # head's stationary slice can be 128 columns wide -> fast weight load)]
            VW = HC * (DH + 1)
            v_sb = apool.tile([P, TT, VW + P - (DH + 1)], bf16, tag="v")
            # attn^T accumulator, same layout as q_sb
            attn_sb = apool.tile([P, 2, S], bf16, tag="attn")
            v_heads = v_sb[:, :, 0:VW].rearrange("p tt (h c) -> p tt h c", c=DH + 1)
            ones_f32 = wpool.tile([P, TT, HC], f32, tag="ones")

            def issue_x(dram, c, nm):
                t = xpool.tile([P, KO, CH], bf16, tag="x", name=f"{nm}{c}")
                nc.sync.dma_start(out=t, in_=dram.ap()[c])
                return t

            # DMA issue order is program order on the sync queue: the first
            # matmul needs only wk + the first k chunk, so those go first and
            # everything else trails in consumption order.
            nc.scalar.dma_start(out=wk_sb, in_=wk.ap())
            nc.scalar.dma_start(out=bk_sb, in_=bk.ap().rearrange("(mo p) -> p mo", p=P))
            # first k chunk in two halves: the ko<4 matmuls start as soon as
            # the first 512KB lands (subtile deps), halving the startup wait
            xk0 = issue_x(xk_d, 0, "xk")
            xk1 = issue_x(xk_d, 1, "xk")
            nc.scalar.dma_start(out=id_sb, in_=ident.ap())
            nc.scalar.dma_start(out=wq_sb, in_=wq.ap())
            nc.scalar.dma_start(out=bq_sb, in_=bq.ap().rearrange("(mo p) -> p mo", p=P))
            nc.vector.memset(ones_f32, 1.0)
            nc.vector.tensor_copy(out=v_heads[:, :, :, DH], in_=ones_f32)
            nc.vector.memset(v_sb[:, :, VW:], 0.0)

            def emit_kq_chunk(x, w_sb, b_sb, dst, c):
                sl = slice(c * CH, (c + 1) * CH)
                for m in range(2):
                    ps = shared_ps.tile([P, CH], f32, tag="ps")
                    for ko in range(KO):
                        nc.tensor.matmul(ps, w_sb[:, ko, ts(m, P)], x[:, ko, :],
                                         start=(ko == 0), stop=(ko == KO - 1))
                    nc.vector.tensor_scalar_add(out=dst[:, m, sl], in0=ps,
                                                scalar1=b_sb[:, m : m + 1])

            # ---- phase 1: projections ----
            # k first (attention needs ALL of k before its first chunk), then
            # q, then v (v is only consumed by PV, which trails exp anyway).
            # x-chunk DMAs are issued one chunk ahead of their consumption.
            emit_kq_chunk(xk0, wk_sb, bk_sb, k_sb, 0)
            xk2 = issue_x(xk_d, 2, "xk")
            emit_kq_chunk(xk1, wk_sb, bk_sb, k_sb, 1)
            xk3 = issue_x(xk_d, 3, "xk")
            emit_kq_chunk(xk2, wk_sb, bk_sb, k_sb, 2)
            xq0 = issue_x(xq_d, 0, "xq")
            emit_kq_chunk(xk3, wk_sb, bk_sb, k_sb, 3)
            xq1 = issue_x(xq_d, 1, "xq")
            emit_kq_chunk(xq0, wq_sb, bq_sb, q_sb, 0)
            xq2 = issue_x(xq_d, 2, "xq")
            nc.scalar.dma_start(out=wv_sb, in_=wv.ap())
            nc.scalar.dma_start(out=bv_row[0:1, :], in_=bv.ap().rearrange("(a d) -> a d", a=1))
            nc.gpsimd.partition_broadcast(bv_bc, bv_row[0:1, :])
            emit_kq_chunk(xq1, wq_sb, bq_sb, q_sb, 1)
            xq3 = issue_x(xq_d, 3, "xq")
            emit_kq_chunk(xq2, wq_sb, bq_sb, q_sb, 2)
            xv0 = issue_x(xv_d, 0, "xv")
            emit_kq_chunk(xq3, wq_sb, bq_sb, q_sb, 3)
            xv1 = issue_x(xv_d, 1, "xv")
            nc.scalar.dma_start(out=wo_sb, in_=wo.ap())

            # V projection: compute v^T (full-width 512-col matmuls), then PE
            # transposes into the natural [s, vdim] layout + bias add. The
            # transposes for (c, m) are emitted one accumulation-slot later so
            # the PE never waits on the DVE evacuation copy.
            xv_t = {0: xv0, 1: xv1}
            pend = None

            def emit_transposes(args):
                vtm, c, m = args
                tp = shared_ps.tile([P, 4, P], bf16, tag="ps", name=f"tp{c}_{m}")
                for th in range(4):
                    nc.tensor.transpose(tp[:, th, :], vtm[:, ts(th, P)], id_sb)
                for th in range(4):
                    nc.vector.tensor_add(
                        out=v_heads[:, 4 * c + th, 2 * m : 2 * m + 2, 0:DH],
                        in0=tp[:, th, :].rearrange("p (h c) -> p h c", c=DH),
                        in1=bv_heads[:, 2 * m : 2 * m + 2, :],
                    )

            for c in range(NCH):
                if c + 2 < NCH:
                    xv_t[c + 2] = issue_x(xv_d, c + 2, "xv")
                for m in range(2):
                    ps = shared_ps.tile([P, CH], f32, tag="ps")
                    for ko in range(KO):
                        nc.tensor.matmul(ps, wv_sb[:, ko, ts(m, P)], xv_t[c][:, ko, :],
                                         start=(ko == 0), stop=(ko == KO - 1))
                    vtm = spool.tile([P, CH], bf16, tag="vt", name=f"vt{c}_{m}")
                    nc.vector.tensor_copy(out=vtm, in_=ps)
                    if pend is not None:
                        emit_transposes(pend)
                    pend = (vtm, c, m)
            emit_transposes(pend)

            def emit_out_proj(c):
                # output projection for query chunk c: 4 s-tiles x 2 n-halves,
                # pairs of PSUM banks batched into one [P, 1024] bf16 DMA
                # (the last chunk DMAs each half separately to shorten the tail)
                for st in range(4 * c, 4 * c + 4):
                    ot = opool.tile([P, D], bf16, tag="ot")
                    for n in range(2):
                        pw = shared_ps.tile([P, 512], f32, tag="ps")
                        for ko in range(2):
                            nc.tensor.matmul(pw, attn_sb[:, ko, ts(st, P)],
                                             wo_sb[:, ko, ts(n, 512)],
                                             start=(ko == 0), stop=(ko == 1))
                        if n == 0:
                            nc.vector.tensor_copy(out=ot[:, ts(n, 512)], in_=pw)
                        else:
                            # Copy lives in the same ACT table as Exp (no
                            # reload); splits the evacuation casts across
                            # DVE and the otherwise-idle ACT slack
                            nc.scalar.activation(out=ot[:, ts(n, 512)], in_=pw,
                                                 func=mybir.ActivationFunctionType.Copy)
                        if c == NCH - 1:
                            nc.sync.dma_start(
                                out=out.ap()[ts(st, P), ts(n, 512)],
                                in_=ot[:, ts(n, 512)])
                    if c != NCH - 1:
                        nc.sync.dma_start(out=out.ap()[ts(st, P), :], in_=ot)

            # ---- phase 2: attention (+ interleaved output projection) ----
            # Emit ALL heads' QK+exp before any PV within a chunk; the Tile
            # framework then interleaves QK/PV on the PE at ~1.1us periods
            # paced between PE and ACT. The output projection for chunk c-1
            # is emitted after chunk c's PV so the PE never waits on the
            # normalize chain (DVE/gpsimd), and its DMA overlaps compute.
            for cidx in range(NCH):
                csl = slice(cidx * CH, (cidx + 1) * CH)
                pts = {}
                for h in range(HC):
                    pts[h] = ptpool.tile([P, TT, CH], bf16, tag="pt",
                                         name=f"pt_c{cidx}_h{h}")
                    base = DH * (h % 2)
                    m = h // 2
                    for tb in range(TT // 2):
                        ps = l_ps.tile([P, 2, CH], f32, tag="l")
                        for j in range(2):
                            tt = 2 * tb + j
                            nc.tensor.matmul(
                                ps[:, j, :],
                                k_sb[base : base + DH, m, ts(tt, P)],
                                q_sb[base : base + DH, m, csl],
                                start=True, stop=True,
                            )
                        nc.scalar.activation(out=pts[h][:, 2 * tb : 2 * tb + 2, :],
                                             in_=ps, func=EXP)
                # PV (+ denominator via the ones column)
                for h in range(HC):
                    base = DH * (h % 2)
                    m = h // 2
                    po = shared_ps.tile([P, CH], f32, tag="ps")
                    for tt in range(TT):
                        nc.tensor.matmul(
                            po[0 : DH + 1, :],
                            v_heads[:, tt, h, :],
                            pts[h][:, tt, :],
                            start=(tt == 0), stop=(tt == TT - 1),
                        )
                    # evacuate PSUM fast (one wide copy), then normalize
                    # from SBUF so the bank frees for the next head's PV
                    tmp = spool.tile([P, CH], f32, tag="tmp")
                    nc.vector.tensor_copy(out=tmp[0 : DH + 1, :], in_=po[0 : DH + 1, :])
                    den = spool.tile([P, CH], f32, tag="den")
                    rec = spool.tile([P, CH], f32, tag="rec")
                    # custom-DVE op requires base partition 0 on both
                    # operands: stage the denominator row down first
                    nc.vector.tensor_copy(out=den[0:1, :], in_=tmp[DH : DH + 1, :])
                    nc.vector.reciprocal_approx_fast(out=rec[0:1, :], in_=den[0:1, :])
                    bc = spool.tile([P, CH], f32, tag="bc")
                    nc.gpsimd.partition_broadcast(bc[0:DH, :], rec[0:1, :])
                    nc.vector.tensor_mul(
                        out=attn_sb[base : base + DH, m, csl],
                        in0=tmp[0:DH, :], in1=bc[0:DH, :],
                    )
                if cidx > 0:
                    emit_out_proj(cidx - 1)
            emit_out_proj(NCH - 1)

    nc.finalize()
    return nc


def kernel(**inputs):
    global _compiled, last_results
    if _compiled is None:
        _compiled = _build()
    nc = _compiled

    query = np.asarray(inputs["query"], np.float32)
    key = np.asarray(inputs["key"], np.float32)
    value = np.asarray(inputs["value"], np.float32)
    Wq = np.asarray(inputs["Wq"], np.float32)
    Wk = np.asarray(inputs["Wk"], np.float32)
    Wv = np.asarray(inputs["Wv"], np.float32)
    Wo = np.asarray(inputs["Wo"], np.float32)
    bq_f = np.asarray(inputs["bq"], np.float32)
    bk_f = np.asarray(inputs["bk"], np.float32)
    bv_f = np.asarray(inputs["bv"], np.float32)
    bo_f = np.asarray(inputs["bo"], np.float32)

    bf = ml_dtypes.bfloat16
    scale = 1.0 / np.sqrt(np.float32(DH))

    def tile_x(x):
        # [S, D] -> [NCH, P, KO, CH]: (c, p, ko, ch) = x[c*CH+ch, ko*P+p]
        return np.ascontiguousarray(
            x.reshape(NCH, CH, KO, P).transpose(0, 3, 2, 1)
        ).astype(bf)

    def tile_w(w):
        # [D, DHC] -> [P, KO, DHC]
        return np.ascontiguousarray(w.reshape(KO, P, DHC).transpose(1, 0, 2)).astype(bf)

    xq = [tile_x(query[b]) for b in range(B)]
    xk = [tile_x(key[b]) for b in range(B)]
    xv = [tile_x(value[b]) for b in range(B)]
    ident = np.eye(P, dtype=bf)

    in_maps = []
    for c in range(NCORES):
        b = c // 4
        sh = c % 4
        sl = slice(DHC * sh, DHC * (sh + 1))
        in_maps.append({
            "xq": xq[b], "xk": xk[b], "xv": xv[b],
            "wq": tile_w(Wq[:, sl] * scale),
            "wk": tile_w(Wk[:, sl]),
            "wv": tile_w(Wv[:, sl]),
            "wo": np.ascontiguousarray(
                Wo[sl, :].reshape(2, P, D).transpose(1, 0, 2)).astype(bf),
            "bq": np.ascontiguousarray(bq_f[sl]) * scale,
            "bk": np.ascontiguousarray(bk_f[sl]),
            "bv": np.ascontiguousarray(bv_f[sl]),
            "ident": ident,
        })

    res = bass_utils.run_bass_kernel_spmd(nc, in_maps, core_ids=list(range(NCORES)))
    last_results = res

    final = np.empty((B, S, D), np.float32)
    for b in range(B):
        acc = res.results[4 * b]["out"].astype(np.float32)
        for sh in range(1, 4):
            acc = acc + res.results[4 * b + sh]["out"].astype(np.float32)
        final[b] = acc + bo_f
    return final


# revision 21
# speedup vs baseline: 1.0052x; 1.0052x over previous
"""Multi-head attention (B=2, S=2048, D=1024, H=16, dh=64) on 8 TRN2 NeuronCores.

Sharding: data-parallel over batch (2) x tensor-parallel over heads (4 per core).
Core c handles batch c//4 and heads [4*(c%4), 4*(c%4)+4). Each core computes a
partial output (its heads' contribution through Wo); the host sums the 4 partials
per batch and adds bo (the unshard step for a sum-sharded tensor).

Matmuls run in bf16 with f32 PSUM accumulation. All DRAM operands are pre-tiled
on host so every DMA line is >=4KB contiguous per partition (small-line DMA ran
at ~140GB/s and dominated the startup critical path).

The PV matmul carries an extra ones-column in the stationary operand so the
softmax denominator falls out of the same accumulation for free; bv is pre-added
to V. Normalization is one approx-reciprocal (custom DVE op, needs base
partition 0) + gpsimd partition broadcast + DVE multiply. The V projection is
computed transposed (512-wide matmuls, full PE rate) and moved to its natural
layout with PE transposes against a host-supplied identity. The output
projection for chunk c is emitted after chunk c+1's attention so the in-order
PE queue never waits on the normalize chain, and the output DMA (bf16) overlaps
remaining compute.
"""

import sys

if "/opt/trn_rl_repo" not in sys.path:
    sys.path.insert(0, "/opt/trn_rl_repo")

import ml_dtypes
import numpy as np

import concourse.bass as bass
import concourse.mybir as mybir
import concourse.tile as tile
from concourse import bacc, bass_utils
from concourse.bass import ts

# Problem constants (hardcoded per contract)
B, S, D = 2, 2048, 1024
H, DH = 16, 64            # total heads, head dim
HC = 4                    # heads per core
DHC = HC * DH             # 256 projected dims per core
NCORES = 8
P = 128
CH = 512                  # query-chunk for attention / projection sub-chunk
NCH = S // CH             # 4
TT = S // P               # 16 key tiles
KO = D // P               # 8 contraction tiles for projections

f32 = mybir.dt.float32
bf16 = mybir.dt.bfloat16
EXP = mybir.ActivationFunctionType.Exp

_compiled = None          # cached nc across calls
last_results = None       # BassKernelResults of the most recent run (for profiling)


def _build():
    nc = bacc.Bacc("TRN2", target_bir_lowering=False, debug=False)

    # Per-core DRAM parameters, pre-tiled on host:
    #   activations x{q,k,v}: [NCH, P, KO, CH]  (per partition: KO*CH*2B = 8KB)
    #   weights w{q,k,v}:     [P, KO, DHC]      (4KB lines)
    #   wo:                   [P, DHC//P, D]    (4KB lines)
    xq_d = nc.dram_tensor("xq", [NCH, P, KO, CH], bf16, kind="ExternalInput")
    xk_d = nc.dram_tensor("xk", [NCH, P, KO, CH], bf16, kind="ExternalInput")
    xv_d = nc.dram_tensor("xv", [NCH, P, KO, CH], bf16, kind="ExternalInput")
    wq = nc.dram_tensor("wq", [P, KO, DHC], bf16, kind="ExternalInput")
    wk = nc.dram_tensor("wk", [P, KO, DHC], bf16, kind="ExternalInput")
    wv = nc.dram_tensor("wv", [P, KO, DHC], bf16, kind="ExternalInput")
    wo = nc.dram_tensor("wo", [P, DHC // P, D], bf16, kind="ExternalInput")
    bq = nc.dram_tensor("bq", [DHC], f32, kind="ExternalInput")
    bk = nc.dram_tensor("bk", [DHC], f32, kind="ExternalInput")
    bv = nc.dram_tensor("bv", [DHC], f32, kind="ExternalInput")
    ident = nc.dram_tensor("ident", [P, P], bf16, kind="ExternalInput")
    out = nc.dram_tensor("out", [S, D], bf16, kind="ExternalOutput")

    with tile.TileContext(nc) as tc:
        with (
            tc.tile_pool(name="weights", bufs=1) as wpool,
            tc.tile_pool(name="acts", bufs=1) as apool,
            tc.tile_pool(name="xin", bufs=3) as xpool,
            tc.tile_pool(name="pt", bufs=6) as ptpool,
            tc.tile_pool(name="small", bufs=2) as spool,
            tc.tile_pool(name="outs", bufs=3) as opool,
            tc.tile_pool(name="shared_ps", bufs=2, space="PSUM") as shared_ps,
            tc.tile_pool(name="l_ps", bufs=3, space="PSUM") as l_ps,
        ):
            # ---- weights / activation tiles ----
            wq_sb = wpool.tile([P, KO, DHC], bf16, tag="wq")
            wk_sb = wpool.tile([P, KO, DHC], bf16, tag="wk")
            wv_sb = wpool.tile([P, KO, DHC], bf16, tag="wv")
            wo_sb = wpool.tile([P, DHC // P, D], bf16, tag="wo")
            id_sb = wpool.tile([P, P], bf16, tag="ident")
            bq_sb = wpool.tile([P, 2], f32, tag="bq")
            bk_sb = wpool.tile([P, 2], f32, tag="bk")
            bv_row = wpool.tile([P, DHC], f32, tag="bv_row")
            bv_bc = wpool.tile([P, DHC], f32, tag="bv_bc")
            bv_heads = bv_bc[:, :].rearrange("p (h c) -> p h c", c=DH)

            # q^T/k^T: [P, m, S] where projected dim r lives at (r % 128, r // 128)
            q_sb = apool.tile([P, 2, S], bf16, tag="q")
            k_sb = apool.tile([P, 2, S], bf16, tag="k")
            # v natural + ones column per head: [P, tt, 4*65 (+63 pad so every
            # head's stationary slice can be 128 columns wide -> fast weight load)]
            VW = HC * (DH + 1)
            v_sb = apool.tile([P, TT, VW + P - (DH + 1)], bf16, tag="v")
            # attn^T accumulator, same layout as q_sb
            attn_sb = apool.tile([P, 2, S], bf16, tag="attn")
            v_heads = v_sb[:, :, 0:VW].rearrange("p tt (h c) -> p tt h c", c=DH + 1)
            ones_f32 = wpool.tile([P, TT, HC], f32, tag="ones")

            warm_sb = wpool.tile([P, CH], bf16, tag="warm")
            def issue_x(dram, c, nm):
                t = xpool.tile([P, KO, CH], bf16, tag="x", name=f"{nm}{c}")
                nc.sync.dma_start(out=t, in_=dram.ap()[c])
                return t

            # DMA issue order is program order on the sync queue: the first
            # matmul needs only wk + the first k chunk, so those go first and
            # everything else trails in consumption order.
            nc.scalar.dma_start(out=wk_sb, in_=wk.ap())
            nc.scalar.dma_start(out=bk_sb, in_=bk.ap().rearrange("(mo p) -> p mo", p=P))
            # first k chunk in two halves: the ko<4 matmuls start as soon as
            # the first 512KB lands (subtile deps), halving the startup wait
            xk0 = issue_x(xk_d, 0, "xk")
            xk1 = issue_x(xk_d, 1, "xk")
            nc.scalar.dma_start(out=id_sb, in_=ident.ap())
            nc.scalar.dma_start(out=wq_sb, in_=wq.ap())
            nc.scalar.dma_start(out=bq_sb, in_=bq.ap().rearrange("(mo p) -> p mo", p=P))
            nc.vector.memset(ones_f32, 1.0)
            nc.vector.tensor_copy(out=v_heads[:, :, :, DH], in_=ones_f32)
            nc.vector.memset(v_sb[:, :, VW:], 0.0)

            def emit_kq_chunk(x, w_sb, b_sb, dst, c):
                sl = slice(c * CH, (c + 1) * CH)
                for m in range(2):
                    ps = shared_ps.tile([P, CH], f32, tag="ps")
                    for ko in range(KO):
                        nc.tensor.matmul(ps, w_sb[:, ko, ts(m, P)], x[:, ko, :],
                                         start=(ko == 0), stop=(ko == KO - 1))
                    nc.vector.tensor_scalar_add(out=dst[:, m, sl], in0=ps,
                                                scalar1=b_sb[:, m : m + 1])

            # ---- phase 1: projections ----
            # k first (attention needs ALL of k before its first chunk), then
            # q, then v (v is only consumed by PV, which trails exp anyway).
            # x-chunk DMAs are issued one chunk ahead of their consumption.
            emit_kq_chunk(xk0, wk_sb, bk_sb, k_sb, 0)
            xk2 = issue_x(xk_d, 2, "xk")
            emit_kq_chunk(xk1, wk_sb, bk_sb, k_sb, 1)
            xk3 = issue_x(xk_d, 3, "xk")
            emit_kq_chunk(xk2, wk_sb, bk_sb, k_sb, 2)
            xq0 = issue_x(xq_d, 0, "xq")
            emit_kq_chunk(xk3, wk_sb, bk_sb, k_sb, 3)
            xq1 = issue_x(xq_d, 1, "xq")
            emit_kq_chunk(xq0, wq_sb, bq_sb, q_sb, 0)
            xq2 = issue_x(xq_d, 2, "xq")
            nc.scalar.dma_start(out=wv_sb, in_=wv.ap())
            nc.scalar.dma_start(out=bv_row[0:1, :], in_=bv.ap().rearrange("(a d) -> a d", a=1))
            nc.gpsimd.partition_broadcast(bv_bc, bv_row[0:1, :])
            emit_kq_chunk(xq1, wq_sb, bq_sb, q_sb, 1)
            xq3 = issue_x(xq_d, 3, "xq")
            emit_kq_chunk(xq2, wq_sb, bq_sb, q_sb, 2)
            xv0 = issue_x(xv_d, 0, "xv")
            emit_kq_chunk(xq3, wq_sb, bq_sb, q_sb, 3)
            xv1 = issue_x(xv_d, 1, "xv")
            nc.scalar.dma_start(out=wo_sb, in_=wo.ap())

            # V projection: compute v^T (full-width 512-col matmuls), then PE
            # transposes into the natural [s, vdim] layout + bias add. The
            # transposes for (c, m) are emitted one accumulation-slot later so
            # the PE never waits on the DVE evacuation copy.
            xv_t = {0: xv0, 1: xv1}
            pend = None

            def emit_transposes(args):
                vtm, c, m = args
                tp = shared_ps.tile([P, 4, P], bf16, tag="ps", name=f"tp{c}_{m}")
                for th in range(4):
                    nc.tensor.transpose(tp[:, th, :], vtm[:, ts(th, P)], id_sb)
                for th in range(4):
                    nc.vector.tensor_add(
                        out=v_heads[:, 4 * c + th, 2 * m : 2 * m + 2, 0:DH],
                        in0=tp[:, th, :].rearrange("p (h c) -> p h c", c=DH),
                        in1=bv_heads[:, 2 * m : 2 * m + 2, :],
                    )

            for c in range(NCH):
                if c + 2 < NCH:
                    xv_t[c + 2] = issue_x(xv_d, c + 2, "xv")
                for m in range(2):
                    ps = shared_ps.tile([P, CH], f32, tag="ps")
                    for ko in range(KO):
                        nc.tensor.matmul(ps, wv_sb[:, ko, ts(m, P)], xv_t[c][:, ko, :],
                                         start=(ko == 0), stop=(ko == KO - 1))
                    vtm = spool.tile([P, CH], bf16, tag="vt", name=f"vt{c}_{m}")
                    nc.vector.tensor_copy(out=vtm, in_=ps)
                    if pend is not None:
                        emit_transposes(pend)
                    pend = (vtm, c, m)
            emit_transposes(pend)

            def emit_out_proj(c):
                # output projection for query chunk c: 4 s-tiles x 2 n-halves,
                # pairs of PSUM banks batched into one [P, 1024] bf16 DMA
                # (the last chunk DMAs each half separately to shorten the tail)
                for st in range(4 * c, 4 * c + 4):
                    ot = opool.tile([P, D], bf16, tag="ot")
                    for n in range(2):
                        pw = shared_ps.tile([P, 512], f32, tag="ps")
                        for ko in range(2):
                            nc.tensor.matmul(pw, attn_sb[:, ko, ts(st, P)],
                                             wo_sb[:, ko, ts(n, 512)],
                                             start=(ko == 0), stop=(ko == 1))
                        if n == 0 and c < 2:
                            nc.vector.tensor_copy(out=ot[:, ts(n, 512)], in_=pw)
                        else:
                            # Copy lives in the same ACT table as Exp (no
                            # reload); splits the evacuation casts across
                            # DVE and the otherwise-idle ACT slack
                            nc.scalar.activation(out=ot[:, ts(n, 512)], in_=pw,
                                                 func=mybir.ActivationFunctionType.Copy)
                        if c == NCH - 1:
                            nc.sync.dma_start(
                                out=out.ap()[ts(st, P), ts(n, 512)],
                                in_=ot[:, ts(n, 512)])
                    if c != NCH - 1:
                        nc.sync.dma_start(out=out.ap()[ts(st, P), :], in_=ot)

            # ---- phase 2: attention (+ interleaved output projection) ----
            # Emit ALL heads' QK+exp before any PV within a chunk; the Tile
            # framework then interleaves QK/PV on the PE at ~1.1us periods
            # paced between PE and ACT. The output projection for chunk c-1
            # is emitted after chunk c's PV so the PE never waits on the
            # normalize chain (DVE/gpsimd), and its DMA overlaps compute.
            for cidx in range(NCH):
                csl = slice(cidx * CH, (cidx + 1) * CH)
                pts = {}
                for h in range(HC):
                    pts[h] = ptpool.tile([P, TT, CH], bf16, tag="pt",
                                         name=f"pt_c{cidx}_h{h}")
                    base = DH * (h % 2)
                    m = h // 2
                    for tb in range(TT // 2):
                        ps = l_ps.tile([P, 2, CH], f32, tag="l")
                        for j in range(2):
                            tt = 2 * tb + j
                            nc.tensor.matmul(
                                ps[:, j, :],
                                k_sb[base : base + DH, m, ts(tt, P)],
                                q_sb[base : base + DH, m, csl],
                                start=True, stop=True,
                            )
                        nc.scalar.activation(out=pts[h][:, 2 * tb : 2 * tb + 2, :],
                                             in_=ps, func=EXP)
                # PV (+ denominator via the ones column)
                for h in range(HC):
                    base = DH * (h % 2)
                    m = h // 2
                    po = shared_ps.tile([P, CH], f32, tag="ps")
                    for tt in range(TT):
                        nc.tensor.matmul(
                            po[0 : DH + 1, :],
                            v_heads[:, tt, h, :],
                            pts[h][:, tt, :],
                            start=(tt == 0), stop=(tt == TT - 1),
                        )
                    # evacuate PSUM fast (one wide copy), then normalize
                    # from SBUF so the bank frees for the next head's PV
                    tmp = spool.tile([P, CH], f32, tag="tmp")
                    nc.vector.tensor_copy(out=tmp[0 : DH + 1, :], in_=po[0 : DH + 1, :])
                    den = spool.tile([P, CH], f32, tag="den")
                    rec = spool.tile([P, CH], f32, tag="rec")
                    # custom-DVE op requires base partition 0 on both
                    # operands: stage the denominator row down first
                    nc.vector.tensor_copy(out=den[0:1, :], in_=tmp[DH : DH + 1, :])
                    nc.vector.reciprocal_approx_fast(out=rec[0:1, :], in_=den[0:1, :])
                    bc = spool.tile([P, CH], f32, tag="bc")
                    nc.gpsimd.partition_broadcast(bc[0:DH, :], rec[0:1, :])
                    nc.vector.tensor_mul(
                        out=attn_sb[base : base + DH, m, csl],
                        in0=tmp[0:DH, :], in1=bc[0:DH, :],
                    )
                if cidx > 0:
                    emit_out_proj(cidx - 1)
            emit_out_proj(NCH - 1)

    nc.finalize()
    return nc


def kernel(**inputs):
    global _compiled, last_results
    if _compiled is None:
        _compiled = _build()
    nc = _compiled

    query = np.asarray(inputs["query"], np.float32)
    key = np.asarray(inputs["key"], np.float32)
    value = np.asarray(inputs["value"], np.float32)
    Wq = np.asarray(inputs["Wq"], np.float32)
    Wk = np.asarray(inputs["Wk"], np.float32)
    Wv = np.asarray(inputs["Wv"], np.float32)
    Wo = np.asarray(inputs["Wo"], np.float32)
    bq_f = np.asarray(inputs["bq"], np.float32)
    bk_f = np.asarray(inputs["bk"], np.float32)
    bv_f = np.asarray(inputs["bv"], np.float32)
    bo_f = np.asarray(inputs["bo"], np.float32)

    bf = ml_dtypes.bfloat16
    scale = 1.0 / np.sqrt(np.float32(DH))

    def tile_x(x):
        # [S, D] -> [NCH, P, KO, CH]: (c, p, ko, ch) = x[c*CH+ch, ko*P+p]
        return np.ascontiguousarray(
            x.reshape(NCH, CH, KO, P).transpose(0, 3, 2, 1)
        ).astype(bf)

    def tile_w(w):
        # [D, DHC] -> [P, KO, DHC]
        return np.ascontiguousarray(w.reshape(KO, P, DHC).transpose(1, 0, 2)).astype(bf)

    xq = [tile_x(query[b]) for b in range(B)]
    xk = [tile_x(key[b]) for b in range(B)]
    xv = [tile_x(value[b]) for b in range(B)]
    ident = np.eye(P, dtype=bf)

    in_maps = []
    for c in range(NCORES):
        b = c // 4
        sh = c % 4
        sl = slice(DHC * sh, DHC * (sh + 1))
        in_maps.append({
            "xq": xq[b], "xk": xk[b], "xv": xv[b],
            "wq": tile_w(Wq[:, sl] * scale),
            "wk": tile_w(Wk[:, sl]),
            "wv": tile_w(Wv[:, sl]),
            "wo": np.ascontiguousarray(
                Wo[sl, :].reshape(2, P, D).transpose(1, 0, 2)).astype(bf),
            "bq": np.ascontiguousarray(bq_f[sl]) * scale,
            "bk": np.ascontiguousarray(bk_f[sl]),
            "bv": np.ascontiguousarray(bv_f[sl]),
            "ident": ident,
        })

    res = bass_utils.run_bass_kernel_spmd(nc, in_maps, core_ids=list(range(NCORES)))
    last_results = res

    final = np.empty((B, S, D), np.float32)
    for b in range(B):
        acc = res.results[4 * b]["out"].astype(np.float32)
        for sh in range(1, 4):
            acc = acc + res.results[4 * b + sh]["out"].astype(np.float32)
        final[b] = acc + bo_f
    return final
